# revision 34
# baseline (speedup 1.0000x reference)
"""Trainium2 Bass kernel for the char-CNN NLP model (data-parallel over 8 cores).

Pipeline:
  host:   emb = x @ emb_w (one-hot projection), laid out [cin, batch, seq]
  device: 3 parallel 1-D conv banks (k=2,3,4; 256 filters each) as float16
          matmuls (fp32 PSUM accumulate); per (channel, batch) max & min over
          sequence; per channel sum of squares -> tiny stats tensor per core
  host:   batchnorm statistics from the factorized mean + device sumsq,
          monotone-affine BN+ReLU+maxpool reconstruction from max/min,
          fc1 -> bn -> relu -> fc2 -> softmax (trivial FLOPs)

BN(c+bias) is affine per channel, so max_t relu(bn(c)) = relu(s*M + t) with
M = max_t c if s>=0 else min_t c — exact, and the conv bias cancels inside BN.
"""

import os
import numpy as np

# ---------------- problem constants (hardcoded per contract) ----------------
B, S, W, V, E = 128, 128, 16, 128, 32
FILTERS = [256, 256, 256]
KS = [2, 3, 4]
NCLS = 10
EPS = 1e-5
NCORES = 8
BL = B // NCORES            # 16 batches per core
CIN = W * E                 # 512 conv input channels
NCC = CIN // 128            # 4 contraction chunks
LS = [S - k + 1 for k in KS]  # 127, 126, 125 valid conv positions
LMM = list(LS)              # fp16 matmul has no even-count restriction
SP = 130                    # padded per-batch stride (keeps DMA rows aligned)
NQ = 4                      # batch quads per core (4 batches each)
GROUPS = [(bank, fc) for bank in range(3) for fc in range(2)]  # 6 (bank, f_chunk)
STATS_COLS = 37             # 16 max | 16 min | 5 sumsq slots

_CACHE = {}
_LAST_RESULTS = None


def _group_tiles(bank):
    return [(cc, kk) for cc in range(NCC) for kk in range(KS[bank])]


def _weight_tile_count():
    return sum(len(_group_tiles(bank)) for bank, _ in GROUPS)


def _build_bass():
    import concourse.tile as tile
    from concourse import bacc, mybir
    from contextlib import ExitStack

    nc = bacc.Bacc("TRN2", target_bir_lowering=False, debug=False, enable_asserts=False)

    ntiles = _weight_tile_count()  # 72
    # float16 transport + compute: 11-bit mantissa (same precision class as
    # float32r) at half the DMA bytes; PE accumulates in fp32.
    emb_d = nc.dram_tensor(
        "emb", [NCC, 128, BL * SP], mybir.dt.float16, kind="ExternalInput"
    ).ap()
    wts_d = nc.dram_tensor(
        "wts", [128, ntiles * 128], mybir.dt.float16, kind="ExternalInput"
    ).ap()
    stats_d = nc.dram_tensor(
        "stats", [len(GROUPS), 128, STATS_COLS], mybir.dt.float32, kind="ExternalOutput"
    ).ap()

    with tile.TileContext(nc) as tc, ExitStack() as ctx:
        const_pool = ctx.enter_context(tc.tile_pool(name="const", bufs=1))
        psum_pool = ctx.enter_context(tc.tile_pool(name="psum", bufs=8, space="PSUM"))
        stats_pool = ctx.enter_context(tc.tile_pool(name="stats", bufs=3))
        scr_pool = ctx.enter_context(tc.tile_pool(name="scr", bufs=4))

        # ---- PE warmup: run junk matmuls on a zeroed tile while input DMAs
        # stream, so HAM un-throttles before the real stream starts ----
        warm = const_pool.tile([128, 512], mybir.dt.float16, name="warm")
        nc.vector.memset(warm[:], 0.0)
        wpsum = psum_pool.tile([128, 512], mybir.dt.float32, tag="ps", name="wps")
        for _ in range(14):
            nc.tensor.matmul(
                wpsum[:], warm[:, :128], warm[:], start=True, stop=True
            )

        # ---- load inputs (HWDGE on sync: FIFO order = priority order, each
        # DMA fans out over all 16 SDMA engines at full bandwidth) ----
        emb_sb = [[None, None] for _ in range(NCC)]  # [chunk][half]
        wt_sb = [None] * len(GROUPS)
        bases = []
        base = 0
        for bank, fc in GROUPS:
            bases.append(base)
            base += len(_group_tiles(bank))

        def alloc_wt(g):
            bank, _ = GROUPS[g]
            n = len(_group_tiles(bank))
            t = const_pool.tile(
                [128, n * 128], mybir.dt.float16, tag=f"w{g}", name=f"w{g}"
            )
            wt_sb[g] = t
            return t

        def load_wt_piece(g, t0, t1):
            nc.sync.dma_start(
                wt_sb[g][:, t0 * 128 : t1 * 128],
                wts_d[:, (bases[g] + t0) * 128 : (bases[g] + t1) * 128],
            )

        def load_emb_half(cc, h):
            t = const_pool.tile(
                [128, 8 * SP], mybir.dt.float16, tag=f"emb{cc}_{h}", name=f"emb{cc}_{h}"
            )
            nc.sync.dma_start(t[:], emb_d[cc][:, h * 8 * SP : (h + 1) * 8 * SP])
            emb_sb[cc][h] = t

        # DMA order mirrors group 0's consumption order. Each dispatch costs
        # ~650ns of sequencer time, so pieces are half-chunks (two quads) —
        # few enough dispatches, small enough for an early stream start.
        # The first weight piece covers only chunk 0's two taps, so the very
        # first matmul unblocks on ~0.33MB instead of ~0.86MB.
        alloc_wt(0)
        load_wt_piece(0, 0, 2)
        load_emb_half(0, 0)
        load_emb_half(0, 1)
        load_wt_piece(0, 2, len(_group_tiles(0)))
        for cc in range(1, NCC):
            for h in range(2):
                load_emb_half(cc, h)
        for g in range(1, len(GROUPS)):
            alloc_wt(g)
            load_wt_piece(g, 0, len(_group_tiles(GROUPS[g][0])))

        # ---- conv banks ----
        for g, (bank, fc) in enumerate(GROUPS):
            L, Lm = LS[bank], LMM[bank]
            tiles = _group_tiles(bank)
            wt = wt_sb[g]
            st = stats_pool.tile(
                [128, STATS_COLS], mybir.dt.float32, tag="st", name=f"st{g}"
            )

            def emit_mms(q, b0, nb, ps):
                for i, (cc, kk) in enumerate(tiles):
                    w_ap = wt[:, i * 128 : (i + 1) * 128]
                    src = emb_sb[cc][q // 2][:].rearrange("p (b t) -> p b t", t=SP)
                    rhs = src[:, (q % 2) * 4 + b0 : (q % 2) * 4 + b0 + nb, kk : kk + Lm]
                    # dst flat-contiguous
                    nc.tensor.matmul(
                        ps[:], w_ap, rhs, start=(i == 0), stop=(i == len(tiles) - 1)
                    )

            def emit_evac(bidx, nb, sq_col, ps):
                pv = ps[:].rearrange("p (b t) -> p b t", t=Lm)[:, :, :L]
                nc.vector.tensor_reduce(
                    st[:, bidx : bidx + nb], pv, axis=mybir.AxisListType.X,
                    op=mybir.AluOpType.max,
                )
                nc.vector.tensor_reduce(
                    st[:, 16 + bidx : 16 + bidx + nb], pv,
                    axis=mybir.AxisListType.X, op=mybir.AluOpType.min,
                )
                scr = scr_pool.tile([128, 4 * 128], mybir.dt.float32, tag="scr")
                scr_v = scr[:, : nb * L].rearrange("p (b t) -> p b t", t=L)
                nc.scalar.activation(
                    scr_v, pv,
                    mybir.ActivationFunctionType.Square,
                    accum_out=st[:, sq_col : sq_col + 1],
                )

            if g == len(GROUPS) - 1:
                # last group: quad-at-a-time so the final evacuation only
                # trails the matmul stream by one quad; the final quad runs
                # as two batch-pairs so the very last evacuation is half-size
                for q in range(NQ - 1):
                    ps = psum_pool.tile(
                        [128, 4 * Lm], mybir.dt.float32, tag="ps", name=f"ps{g}_{q}"
                    )
                    emit_mms(q, 0, 4, ps)
                    emit_evac(q * 4, 4, 32 + q, ps)
                for h in range(2):
                    ps = psum_pool.tile(
                        [128, 2 * Lm], mybir.dt.float32, tag="ps", name=f"ps{g}_3{h}"
                    )
                    emit_mms(NQ - 1, h * 2, 2, ps)
                    emit_evac(12 + h * 2, 2, 35 + h, ps)
            else:
                psums = [
                    psum_pool.tile(
                        [128, 4 * Lm], mybir.dt.float32, tag="ps", name=f"ps{g}_{q}"
                    )
                    for q in range(NQ)
                ]
                for i, (cc, kk) in enumerate(tiles):
                    w_ap = wt[:, i * 128 : (i + 1) * 128]
                    first, last = i == 0, i == len(tiles) - 1
                    for q in range(NQ):
                        src = emb_sb[cc][q // 2][:].rearrange(
                            "p (b t) -> p b t", t=SP
                        )
                        rhs = src[:, (q % 2) * 4 : (q % 2) * 4 + 4, kk : kk + Lm]
                        nc.tensor.matmul(
                            psums[q][:], w_ap, rhs, start=first, stop=last
                        )
                for q in range(NQ):
                    emit_evac(q * 4, 4, 32 + q, psums[q])
            # last group's stats go out on the ACT HWDGE ring: no cross-engine
            # hop after the final Square, and the sync ring's input FIFO is
            # not in the way
            if g == len(GROUPS) - 1:
                nc.scalar.dma_start(stats_d[g], st[:])
            else:
                nc.sync.dma_start(stats_d[g], st[:])

    nc.compile()
    return nc


def _get_compiled():
    if "nc" not in _CACHE:
        _CACHE["nc"] = _build_bass()
    return _CACHE["nc"]


def _maybe_enable_trace():
    if os.environ.get("KERNEL_TRACE") != "1":
        return False
    try:
        import sys, types

        if "antenv.axon_hooks" not in sys.modules:
            mod = types.ModuleType("antenv.axon_hooks")
            _h = {"hook": None}
            mod.set_axon_ntff_profile_hook = lambda h: _h.__setitem__("hook", h)
            mod.get_axon_ntff_profile_hook = lambda: _h["hook"]
            sys.modules["antenv.axon_hooks"] = mod
            import antenv

            antenv.axon_hooks = mod
            from trn_agent_boot.trn_boot import _ntff_profile_via_ctypes

            mod.set_axon_ntff_profile_hook(
                _ntff_profile_via_ctypes("/opt/axon/libaxon_pjrt.so")
            )
        import concourse.bass_utils as bu

        bu.upload_artifacts = lambda tmpdir: tmpdir
        return True
    except Exception:
        return False


def kernel(
    x, emb_w,
    conv_w0, conv_b0, bn_g0, bn_b0,
    conv_w1, conv_b1, bn_g1, bn_b1,
    conv_w2, conv_b2, bn_g2, bn_b2,
    fc1_w, fc1_b, bn1_g, bn1_b, fc2_w, fc2_b,
):
    global _LAST_RESULTS
    from concourse.bass_utils import run_bass_kernel_spmd

    x = np.asarray(x, dtype=np.float32)
    emb_w = np.asarray(emb_w, dtype=np.float32)
    conv_ws = [np.asarray(w, dtype=np.float32) for w in (conv_w0, conv_w1, conv_w2)]
    bn_gs = [np.asarray(v, dtype=np.float64) for v in (bn_g0, bn_g1, bn_g2)]
    bn_bs = [np.asarray(v, dtype=np.float64) for v in (bn_b0, bn_b1, bn_b2)]

    # ---- host: embedding (x is one-hot in practice; dense matmul is exact) ----
    e = x.reshape(-1, V) @ emb_w                       # [B*S*W, E]
    e = e.reshape(B, S, CIN)                           # [B, S, 512]
    embT = np.ascontiguousarray(e.transpose(2, 0, 1))  # [512, B, S]

    # ---- pack device inputs ----
    in_maps = []
    ntiles = _weight_tile_count()
    wts = np.empty((128, ntiles * 128), dtype=np.float16)
    i = 0
    for bank, fc in GROUPS:
        cw = conv_ws[bank]                             # [256, 512, k]
        for cc, kk in _group_tiles(bank):
            tile_fm = cw[fc * 128 : (fc + 1) * 128, cc * 128 : (cc + 1) * 128, kk]
            wts[:, i * 128 : (i + 1) * 128] = tile_fm.T   # [ci, f]
            i += 1
    embP = np.zeros((CIN, B, SP), dtype=np.float16)
    embP[:, :, :S] = embT
    for c in range(NCORES):
        shard = np.ascontiguousarray(
            embP[:, c * BL : (c + 1) * BL, :]
        ).reshape(NCC, 128, BL * SP)
        in_maps.append({"emb": shard, "wts": wts})

    nc = _get_compiled()
    trace = _maybe_enable_trace()
    res = run_bass_kernel_spmd(
        nc, in_maps, core_ids=list(range(NCORES)), trace=trace,
        tmpdir=os.environ.get("KERNEL_TRACE_DIR") or None,
    )
    _LAST_RESULTS = res

    # ---- host: combine stats -> BN -> pooled -> fc head (float64) ----
    FT = sum(FILTERS)  # 768
    cmax = np.empty((FT, B), dtype=np.float64)
    cmin = np.empty((FT, B), dtype=np.float64)
    sumsq = np.zeros(FT, dtype=np.float64)
    for c in range(NCORES):
        stats = res.results[c]["stats"].astype(np.float64)  # [6, 128, 36]
        for g, (bank, fc) in enumerate(GROUPS):
            ch = bank * 256 + fc * 128
            sl = slice(ch, ch + 128)
            bs = slice(c * BL, (c + 1) * BL)
            cmax[sl, bs] = stats[g, :, 0:16]
            cmin[sl, bs] = stats[g, :, 16:32]
            hi = 37 if g == len(GROUPS) - 1 else 36
            sumsq[sl] += stats[g, :, 32:hi].sum(axis=1)

    # channel means via the factorized sum (exact: sum_t conv = w . window-sums)
    embT64 = embT.astype(np.float64)
    st_sum = embT64.sum(axis=1)                        # [512, S] summed over batch
    cum = np.concatenate(
        [np.zeros((CIN, 1)), np.cumsum(st_sum, axis=1)], axis=1
    )                                                  # [512, S+1]
    mean = np.empty(FT, dtype=np.float64)
    for bank in range(3):
        k, L = KS[bank], LS[bank]
        cw = conv_ws[bank].astype(np.float64)          # [256, 512, k]
        hs = np.stack([cum[:, kk + L] - cum[:, kk] for kk in range(k)], axis=1)
        mean[bank * 256 : (bank + 1) * 256] = (
            np.einsum("fck,ck->f", cw, hs) / (B * L)
        )

    counts = np.repeat([B * L for L in LS], FILTERS)
    var = sumsq / counts - mean * mean
    g_all = np.concatenate(bn_gs)
    b_all = np.concatenate(bn_bs)
    s = g_all / np.sqrt(var + EPS)
    shift = b_all - mean * s
    M = np.where(s[:, None] >= 0.0, cmax, cmin)        # [768, B]
    pooled = np.maximum(s[:, None] * M + shift[:, None], 0.0).T  # [B, 768]

    z = pooled @ np.asarray(fc1_w, dtype=np.float64) + np.asarray(
        fc1_b, dtype=np.float64
    )
    mu = z.mean(axis=0, keepdims=True)
    vz = np.square(z - mu).mean(axis=0, keepdims=True)
    z = (z - mu) / np.sqrt(vz + EPS) * np.asarray(
        bn1_g, dtype=np.float64
    ) + np.asarray(bn1_b, dtype=np.float64)
    z = np.maximum(z, 0.0)
    logits = z @ np.asarray(fc2_w, dtype=np.float64) + np.asarray(
        fc2_b, dtype=np.float64
    )
    logits -= logits.max(axis=1, keepdims=True)
    p = np.exp(logits)
    p /= p.sum(axis=1, keepdims=True)
    return p.astype(np.float32)


# revision 35
# speedup vs baseline: 1.1835x; 1.1835x over previous
"""Trainium2 Bass kernel for the char-CNN NLP model (data-parallel over 8 cores).

Pipeline:
  host:   emb = x @ emb_w (one-hot projection), laid out [cin, batch, seq]
  device: 3 parallel 1-D conv banks (k=2,3,4; 256 filters each) as float16
          matmuls (fp32 PSUM accumulate); per (channel, batch) max & min over
          sequence; per channel sum of squares -> tiny stats tensor per core
  host:   batchnorm statistics from the factorized mean + device sumsq,
          monotone-affine BN+ReLU+maxpool reconstruction from max/min,
          fc1 -> bn -> relu -> fc2 -> softmax (trivial FLOPs)

BN(c+bias) is affine per channel, so max_t relu(bn(c)) = relu(s*M + t) with
M = max_t c if s>=0 else min_t c — exact, and the conv bias cancels inside BN.
"""

import os
import numpy as np

# ---------------- problem constants (hardcoded per contract) ----------------
B, S, W, V, E = 128, 128, 16, 128, 32
FILTERS = [256, 256, 256]
KS = [2, 3, 4]
NCLS = 10
EPS = 1e-5
NCORES = 8
BL = B // NCORES            # 16 batches per core
CIN = W * E                 # 512 conv input channels
NCC = CIN // 128            # 4 contraction chunks
LS = [S - k + 1 for k in KS]  # 127, 126, 125 valid conv positions
LMM = list(LS)              # fp16 matmul has no even-count restriction
SP = 130                    # padded per-batch stride (keeps DMA rows aligned)
NQ = 4                      # batch quads per core (4 batches each)
GROUPS = [(bank, fc) for bank in range(3) for fc in range(2)]  # 6 (bank, f_chunk)
STATS_COLS = 37             # 16 max | 16 min | 5 sumsq slots

_CACHE = {}
_LAST_RESULTS = None


def _group_tiles(bank):
    return [(cc, kk) for cc in range(NCC) for kk in range(KS[bank])]


def _weight_tile_count():
    return sum(len(_group_tiles(bank)) for bank, _ in GROUPS)


def _build_bass():
    import concourse.tile as tile
    from concourse import bacc, mybir
    from contextlib import ExitStack

    nc = bacc.Bacc("TRN2", target_bir_lowering=False, debug=False, enable_asserts=False)

    ntiles = _weight_tile_count()  # 72
    # float16 transport + compute: 11-bit mantissa (same precision class as
    # float32r) at half the DMA bytes; PE accumulates in fp32.
    emb_d = nc.dram_tensor(
        "emb", [NCC, 128, BL * SP], mybir.dt.float16, kind="ExternalInput"
    ).ap()
    wts_d = nc.dram_tensor(
        "wts", [128, ntiles * 128], mybir.dt.float16, kind="ExternalInput"
    ).ap()
    stats_d = nc.dram_tensor(
        "stats", [len(GROUPS), 128, STATS_COLS], mybir.dt.float32, kind="ExternalOutput"
    ).ap()

    with tile.TileContext(nc) as tc, ExitStack() as ctx:
        const_pool = ctx.enter_context(tc.tile_pool(name="const", bufs=1))
        psum_pool = ctx.enter_context(tc.tile_pool(name="psum", bufs=8, space="PSUM"))
        stats_pool = ctx.enter_context(tc.tile_pool(name="stats", bufs=3))
        scr_pool = ctx.enter_context(tc.tile_pool(name="scr", bufs=4))

        # ---- PE warmup: run junk matmuls on a zeroed tile while input DMAs
        # stream, so HAM un-throttles before the real stream starts ----
        warm = const_pool.tile([128, 512], mybir.dt.float16, name="warm")
        nc.vector.memset(warm[:], 0.0)
        wpsum = psum_pool.tile([128, 512], mybir.dt.float32, tag="ps", name="wps")
        for _ in range(14):
            nc.tensor.matmul(
                wpsum[:], warm[:, :128], warm[:], start=True, stop=True
            )

        # ---- load inputs (HWDGE on sync: FIFO order = priority order, each
        # DMA fans out over all 16 SDMA engines at full bandwidth) ----
        emb_sb = [[None, None] for _ in range(NCC)]  # [chunk][half]
        wt_sb = [None] * len(GROUPS)
        bases = []
        base = 0
        for bank, fc in GROUPS:
            bases.append(base)
            base += len(_group_tiles(bank))

        def alloc_wt(g):
            bank, _ = GROUPS[g]
            n = len(_group_tiles(bank))
            t = const_pool.tile(
                [128, n * 128], mybir.dt.float16, tag=f"w{g}", name=f"w{g}"
            )
            wt_sb[g] = t
            return t

        def load_wt_piece(g, t0, t1):
            nc.sync.dma_start(
                wt_sb[g][:, t0 * 128 : t1 * 128],
                wts_d[:, (bases[g] + t0) * 128 : (bases[g] + t1) * 128],
            )

        def load_emb_half(cc, h):
            t = const_pool.tile(
                [128, 8 * SP], mybir.dt.float16, tag=f"emb{cc}_{h}", name=f"emb{cc}_{h}"
            )
            nc.sync.dma_start(t[:], emb_d[cc][:, h * 8 * SP : (h + 1) * 8 * SP])
            emb_sb[cc][h] = t

        # DMA order mirrors group 0's consumption order. Each dispatch costs
        # ~650ns of sequencer time, so pieces are half-chunks (two quads) —
        # few enough dispatches, small enough for an early stream start.
        # The first weight piece covers only chunk 0's two taps, so the very
        # first matmul unblocks on ~0.33MB instead of ~0.86MB.
        alloc_wt(0)
        load_wt_piece(0, 0, 2)
        load_emb_half(0, 0)
        load_emb_half(0, 1)
        load_wt_piece(0, 2, len(_group_tiles(0)))
        for cc in range(1, NCC):
            for h in range(2):
                load_emb_half(cc, h)
        for g in range(1, len(GROUPS)):
            alloc_wt(g)
            load_wt_piece(g, 0, len(_group_tiles(GROUPS[g][0])))

        # ---- conv banks ----
        for g, (bank, fc) in enumerate(GROUPS):
            L, Lm = LS[bank], LMM[bank]
            tiles = _group_tiles(bank)
            wt = wt_sb[g]
            st = stats_pool.tile(
                [128, STATS_COLS], mybir.dt.float32, tag="st", name=f"st{g}"
            )

            def emit_mms(q, b0, nb, ps):
                for i, (cc, kk) in enumerate(tiles):
                    w_ap = wt[:, i * 128 : (i + 1) * 128]
                    src = emb_sb[cc][q // 2][:].rearrange("p (b t) -> p b t", t=SP)
                    rhs = src[:, (q % 2) * 4 + b0 : (q % 2) * 4 + b0 + nb, kk : kk + Lm]
                    # dst flat-contiguous
                    nc.tensor.matmul(
                        ps[:], w_ap, rhs, start=(i == 0), stop=(i == len(tiles) - 1)
                    )

            def emit_evac(bidx, nb, sq_col, ps):
                pv = ps[:].rearrange("p (b t) -> p b t", t=Lm)[:, :, :L]
                nc.vector.tensor_reduce(
                    st[:, bidx : bidx + nb], pv, axis=mybir.AxisListType.X,
                    op=mybir.AluOpType.max,
                )
                nc.vector.tensor_reduce(
                    st[:, 16 + bidx : 16 + bidx + nb], pv,
                    axis=mybir.AxisListType.X, op=mybir.AluOpType.min,
                )
                scr = scr_pool.tile([128, 4 * 128], mybir.dt.float32, tag="scr")
                scr_v = scr[:, : nb * L].rearrange("p (b t) -> p b t", t=L)
                nc.scalar.activation(
                    scr_v, pv,
                    mybir.ActivationFunctionType.Square,
                    accum_out=st[:, sq_col : sq_col + 1],
                )

            if g == len(GROUPS) - 1:
                # last group: quad-at-a-time so the final evacuation only
                # trails the matmul stream by one quad; the final quad runs
                # as two batch-pairs so the very last evacuation is half-size
                for q in range(NQ - 1):
                    ps = psum_pool.tile(
                        [128, 4 * Lm], mybir.dt.float32, tag="ps", name=f"ps{g}_{q}"
                    )
                    emit_mms(q, 0, 4, ps)
                    emit_evac(q * 4, 4, 32 + q, ps)
                for h in range(2):
                    ps = psum_pool.tile(
                        [128, 2 * Lm], mybir.dt.float32, tag="ps", name=f"ps{g}_3{h}"
                    )
                    emit_mms(NQ - 1, h * 2, 2, ps)
                    emit_evac(12 + h * 2, 2, 35 + h, ps)
            else:
                psums = [
                    psum_pool.tile(
                        [128, 4 * Lm], mybir.dt.float32, tag="ps", name=f"ps{g}_{q}"
                    )
                    for q in range(NQ)
                ]
                for i, (cc, kk) in enumerate(tiles):
                    w_ap = wt[:, i * 128 : (i + 1) * 128]
                    first, last = i == 0, i == len(tiles) - 1
                    for q in range(NQ):
                        src = emb_sb[cc][q // 2][:].rearrange(
                            "p (b t) -> p b t", t=SP
                        )
                        rhs = src[:, (q % 2) * 4 : (q % 2) * 4 + 4, kk : kk + Lm]
                        nc.tensor.matmul(
                            psums[q][:], w_ap, rhs, start=first, stop=last
                        )
                for q in range(NQ):
                    emit_evac(q * 4, 4, 32 + q, psums[q])
            nc.sync.dma_start(stats_d[g], st[:])

    nc.compile()
    return nc


def _get_compiled():
    if "nc" not in _CACHE:
        _CACHE["nc"] = _build_bass()
    return _CACHE["nc"]


def _maybe_enable_trace():
    if os.environ.get("KERNEL_TRACE") != "1":
        return False
    try:
        import sys, types

        if "antenv.axon_hooks" not in sys.modules:
            mod = types.ModuleType("antenv.axon_hooks")
            _h = {"hook": None}
            mod.set_axon_ntff_profile_hook = lambda h: _h.__setitem__("hook", h)
            mod.get_axon_ntff_profile_hook = lambda: _h["hook"]
            sys.modules["antenv.axon_hooks"] = mod
            import antenv

            antenv.axon_hooks = mod
            from trn_agent_boot.trn_boot import _ntff_profile_via_ctypes

            mod.set_axon_ntff_profile_hook(
                _ntff_profile_via_ctypes("/opt/axon/libaxon_pjrt.so")
            )
        import concourse.bass_utils as bu

        bu.upload_artifacts = lambda tmpdir: tmpdir
        return True
    except Exception:
        return False


def kernel(
    x, emb_w,
    conv_w0, conv_b0, bn_g0, bn_b0,
    conv_w1, conv_b1, bn_g1, bn_b1,
    conv_w2, conv_b2, bn_g2, bn_b2,
    fc1_w, fc1_b, bn1_g, bn1_b, fc2_w, fc2_b,
):
    global _LAST_RESULTS
    from concourse.bass_utils import run_bass_kernel_spmd

    x = np.asarray(x, dtype=np.float32)
    emb_w = np.asarray(emb_w, dtype=np.float32)
    conv_ws = [np.asarray(w, dtype=np.float32) for w in (conv_w0, conv_w1, conv_w2)]
    bn_gs = [np.asarray(v, dtype=np.float64) for v in (bn_g0, bn_g1, bn_g2)]
    bn_bs = [np.asarray(v, dtype=np.float64) for v in (bn_b0, bn_b1, bn_b2)]

    # ---- host: embedding (x is one-hot in practice; dense matmul is exact) ----
    e = x.reshape(-1, V) @ emb_w                       # [B*S*W, E]
    e = e.reshape(B, S, CIN)                           # [B, S, 512]
    embT = np.ascontiguousarray(e.transpose(2, 0, 1))  # [512, B, S]

    # ---- pack device inputs ----
    in_maps = []
    ntiles = _weight_tile_count()
    wts = np.empty((128, ntiles * 128), dtype=np.float16)
    i = 0
    for bank, fc in GROUPS:
        cw = conv_ws[bank]                             # [256, 512, k]
        for cc, kk in _group_tiles(bank):
            tile_fm = cw[fc * 128 : (fc + 1) * 128, cc * 128 : (cc + 1) * 128, kk]
            wts[:, i * 128 : (i + 1) * 128] = tile_fm.T   # [ci, f]
            i += 1
    embP = np.zeros((CIN, B, SP), dtype=np.float16)
    embP[:, :, :S] = embT
    for c in range(NCORES):
        shard = np.ascontiguousarray(
            embP[:, c * BL : (c + 1) * BL, :]
        ).reshape(NCC, 128, BL * SP)
        in_maps.append({"emb": shard, "wts": wts})

    nc = _get_compiled()
    trace = _maybe_enable_trace()
    res = run_bass_kernel_spmd(
        nc, in_maps, core_ids=list(range(NCORES)), trace=trace,
        tmpdir=os.environ.get("KERNEL_TRACE_DIR") or None,
    )
    _LAST_RESULTS = res

    # ---- host: combine stats -> BN -> pooled -> fc head (float64) ----
    FT = sum(FILTERS)  # 768
    cmax = np.empty((FT, B), dtype=np.float64)
    cmin = np.empty((FT, B), dtype=np.float64)
    sumsq = np.zeros(FT, dtype=np.float64)
    for c in range(NCORES):
        stats = res.results[c]["stats"].astype(np.float64)  # [6, 128, 36]
        for g, (bank, fc) in enumerate(GROUPS):
            ch = bank * 256 + fc * 128
            sl = slice(ch, ch + 128)
            bs = slice(c * BL, (c + 1) * BL)
            cmax[sl, bs] = stats[g, :, 0:16]
            cmin[sl, bs] = stats[g, :, 16:32]
            hi = 37 if g == len(GROUPS) - 1 else 36
            sumsq[sl] += stats[g, :, 32:hi].sum(axis=1)

    # channel means via the factorized sum (exact: sum_t conv = w . window-sums)
    embT64 = embT.astype(np.float64)
    st_sum = embT64.sum(axis=1)                        # [512, S] summed over batch
    cum = np.concatenate(
        [np.zeros((CIN, 1)), np.cumsum(st_sum, axis=1)], axis=1
    )                                                  # [512, S+1]
    mean = np.empty(FT, dtype=np.float64)
    for bank in range(3):
        k, L = KS[bank], LS[bank]
        cw = conv_ws[bank].astype(np.float64)          # [256, 512, k]
        hs = np.stack([cum[:, kk + L] - cum[:, kk] for kk in range(k)], axis=1)
        mean[bank * 256 : (bank + 1) * 256] = (
            np.einsum("fck,ck->f", cw, hs) / (B * L)
        )

    counts = np.repeat([B * L for L in LS], FILTERS)
    var = sumsq / counts - mean * mean
    g_all = np.concatenate(bn_gs)
    b_all = np.concatenate(bn_bs)
    s = g_all / np.sqrt(var + EPS)
    shift = b_all - mean * s
    M = np.where(s[:, None] >= 0.0, cmax, cmin)        # [768, B]
    pooled = np.maximum(s[:, None] * M + shift[:, None], 0.0).T  # [B, 768]

    z = pooled @ np.asarray(fc1_w, dtype=np.float64) + np.asarray(
        fc1_b, dtype=np.float64
    )
    mu = z.mean(axis=0, keepdims=True)
    vz = np.square(z - mu).mean(axis=0, keepdims=True)
    z = (z - mu) / np.sqrt(vz + EPS) * np.asarray(
        bn1_g, dtype=np.float64
    ) + np.asarray(bn1_b, dtype=np.float64)
    z = np.maximum(z, 0.0)
    logits = z @ np.asarray(fc2_w, dtype=np.float64) + np.asarray(
        fc2_b, dtype=np.float64
    )
    logits -= logits.max(axis=1, keepdims=True)
    p = np.exp(logits)
    p /= p.sum(axis=1, keepdims=True)
    return p.astype(np.float32)
